# revision 1
# baseline (speedup 1.0000x reference)
"""Causal self-attention on 8 Trainium2 NeuronCores.

Sharding: core c = 4*b + g handles batch b (of 2) and head-group g (4 of 16
heads).  Weights are column-sliced (Wq/Wk/Wv) / row-sliced (Wp) per head
group; each core returns a partial output [T, C] that the host sums per
batch and biases.

Per-core dataflow (all matmul operands bf16, fp32 PSUM accumulation):
  x^T (host-transposed, with an appended ones-row for the biases)
    Q^T = (Wq|bq)^T-style augmented matmul -> [256, T]   (d on partitions)
    K^T likewise; V = x @ (Wv|bv) -> [T, 260] where every 65th column is the
    constant 1 (ones-column trick: the P@V matmul then also produces the
    softmax denominator as row 64 of each head's output).
  S^T[k,q] = K^T.T @ Q^T per 128k x 512q block (causal blocks only)
  P = exp(0.125*S^T) on ACT straight out of PSUM, diagonal blocks masked
  y_aug^T[d,q] = V.T @ P accumulated over k tiles; row 64 = sum_k P
  y^T = y_aug^T[0:64] * (1/denom)  (denominator broadcast across partitions
        via a rank-1 ones x recip matmul)
  out_partial = y^T.T @ Wp  ->  DMA'd straight from PSUM to DRAM fp32.
"""

import re
import sys

sys.path.insert(0, "/opt/trn_rl_repo")

import numpy as np
import ml_dtypes

import bass_rust
import concourse.bass as bass
import concourse.mybir as mybir
from concourse.tile import TileContext
from concourse.vector_clock import ScopedClock

BF16 = ml_dtypes.bfloat16

T = 2048          # sequence length per batch
C = 1024          # model dim
DHG = 256         # head dims per core (4 heads x 64)
DH = 64           # head dim
NH = 4            # heads per core
DVA = NH * (DH + 1)  # 260: V augmented with a ones-column per head
KT = C // 128     # 8 full contraction tiles
TB = T // 128     # 16 row tiles
QC = T // 512     # 4 query chunks
SCALE = 0.125     # 1/sqrt(64)


class SplitDrainTileContext(TileContext):
    """Walrus TRN2 codegen rejects >4 sync waits on one instruction; the
    stock TileContext exit drain carries one wait per live proc.  Split them
    into single-wait drains chained on the sync sequencer."""

    def _drain_and_barrier(self, tick_clock, wait_clock):
        gc = tick_clock.global_clock
        ticks = [int(x) for x in re.findall(r"\d+", repr(gc))]
        for proc, tick in [(i, t) for i, t in enumerate(ticks) if t > 0]:
            sub = bass_rust.VectorClock()
            sub.require_at_least(proc, tick)
            inst = self.nc.sync.drain()
            wait_clock.add_sem_waits(inst.ins, ScopedClock({None: sub}))
        self.nc.sync.drain()
        self.nc.all_engine_barrier()
        assert self.sems is not None
        popped = self.nc._tile_sem_poison_stack.pop()
        assert popped is self._sem_poison
        self.nc.clear_and_free_semaphores(list(self.sems.allocated().values()))
        self.nc.all_engine_barrier()


def _split_excess_waits(nc, max_waits=1):
    """Walrus TRN2 codegen allows only ~2 sync waits per instruction.
    Hoist any excess onto same-engine InstNoOp carriers placed immediately
    before the instruction — the engine is in-order, so semantics are
    identical."""
    ctr = 0
    for fn in nc.m.functions:
        for bb in fn.blocks:
            new = []
            for inst in bb.instructions:
                si = inst.sync_info
                if (si and si.on_wait and len(si.on_wait) > max_waits
                        and "Unassigned" not in str(inst.engine)):
                    waits = list(si.on_wait)
                    for w in waits[:-max_waits]:
                        ctr += 1
                        nop = bass_rust.InstNoOp(
                            name=f"wsplit-{ctr}", ins=[], outs=[])
                        nop.engine = inst.engine
                        nop.sync_info = bass_rust.SyncInfo(
                            on_wait=[w], on_update=[])
                        new.append(nop)
                    si.on_wait = waits[-max_waits:]
                new.append(inst)
            bb.instructions = new


def build_attention_nc(legalize=True):
    nc = bass.Bass(num_devices=8)
    dt = mybir.dt

    xt = nc.dram_tensor("xt", [C, T], dt.bfloat16, kind="ExternalInput")
    xb = nc.dram_tensor("xb", [1, T], dt.bfloat16, kind="ExternalInput")
    wq = nc.dram_tensor("wq", [C, DHG], dt.bfloat16, kind="ExternalInput")
    wk = nc.dram_tensor("wk", [C, DHG], dt.bfloat16, kind="ExternalInput")
    wv = nc.dram_tensor("wv", [C, DVA], dt.bfloat16, kind="ExternalInput")
    wqb = nc.dram_tensor("wqb", [1, DHG], dt.bfloat16, kind="ExternalInput")
    wkb = nc.dram_tensor("wkb", [1, DHG], dt.bfloat16, kind="ExternalInput")
    wvb = nc.dram_tensor("wvb", [1, DVA], dt.bfloat16, kind="ExternalInput")
    wp = nc.dram_tensor("wp", [DHG, C], dt.bfloat16, kind="ExternalInput")
    mask = nc.dram_tensor("mask", [128, 128], dt.bfloat16, kind="ExternalInput")
    out = nc.dram_tensor("out", [T, C], dt.float32, kind="ExternalOutput")

    with SplitDrainTileContext(nc) as tc:
        with (
            tc.tile_pool(name="weights", bufs=1) as wpool,
            tc.tile_pool(name="acts", bufs=1) as apool,
            tc.tile_pool(name="ptiles", bufs=72) as ppool,
            tc.tile_pool(name="small", bufs=4) as spool,
            tc.tile_pool(name="ps_mm", bufs=2, space="PSUM") as ps_mm,
            tc.tile_pool(name="ps_s", bufs=3, space="PSUM") as ps_s,
            tc.tile_pool(name="ps_y", bufs=2, space="PSUM") as ps_y,
            tc.tile_pool(name="ps_bc", bufs=1, space="PSUM") as ps_bc,
        ):
            # ---- load everything into SBUF, (k-tile x chunk) granular so
            # the first projection matmuls start as soon as slices land ----
            xt_kc = [[apool.tile([128, 512], dt.bfloat16, name=f"xt{kt}_{c}")
                      for c in range(QC)] for kt in range(KT)]
            wq_k = [wpool.tile([128, DHG], dt.bfloat16, name=f"wq{kt}")
                    for kt in range(KT)]
            wk_k = [wpool.tile([128, DHG], dt.bfloat16, name=f"wk{kt}")
                    for kt in range(KT)]
            wv_k = [wpool.tile([128, DVA], dt.bfloat16, name=f"wv{kt}")
                    for kt in range(KT)]
            # small tensors first: the first diagonal mask-mult head-of-line
            # blocks the whole Pool stream if the mask lands late
            mask_sb = wpool.tile([128, 128], dt.bfloat16)
            nc.sync.dma_start(mask_sb[:], mask[:])
            xb_sb = wpool.tile([1, T], dt.bfloat16)
            nc.sync.dma_start(xb_sb[:], xb[:])
            wqb_sb = wpool.tile([1, DHG], dt.bfloat16)
            wkb_sb = wpool.tile([1, DHG], dt.bfloat16)
            wvb_sb = wpool.tile([1, DVA], dt.bfloat16)
            nc.sync.dma_start(wqb_sb[:], wqb[:])
            nc.sync.dma_start(wkb_sb[:], wkb[:])
            nc.sync.dma_start(wvb_sb[:], wvb[:])
            ones64 = wpool.tile([1, DH], dt.bfloat16)
            nc.vector.memset(ones64[:], 1.0)
            # chunk-0-critical loads, then V weights, then the bulk of x^T
            for kt in range(KT):
                nc.sync.dma_start(wq_k[kt][:], wq[kt * 128:(kt + 1) * 128, :])
                nc.sync.dma_start(wk_k[kt][:], wk[kt * 128:(kt + 1) * 128, :])
                nc.sync.dma_start(
                    xt_kc[kt][0][:], xt[kt * 128:(kt + 1) * 128, 0:512])
            for kt in range(KT):
                nc.sync.dma_start(wv_k[kt][:], wv[kt * 128:(kt + 1) * 128, :])
            for c in range(1, QC):
                for kt in range(KT):
                    nc.sync.dma_start(
                        xt_kc[kt][c][:],
                        xt[kt * 128:(kt + 1) * 128, c * 512:(c + 1) * 512])
            wp_sb = wpool.tile([128, 2, C], dt.bfloat16)
            nc.sync.dma_start(wp_sb[:], wp.rearrange("(k p) d -> p k d", p=128))

            # fine-grained tiles so consumers start as soon as their slice
            # is ready (qt/kt/yt: [d-block][t-chunk], v: per t-block)
            qt_sb = [[apool.tile([128, 512], dt.bfloat16, name=f"qt{j}_{c}")
                      for c in range(QC)] for j in range(2)]
            kt_sb = [[apool.tile([128, 512], dt.bfloat16, name=f"kt{j}_{c}")
                      for c in range(QC)] for j in range(2)]
            v_sb = [apool.tile([128, DVA], dt.bfloat16, name=f"v{tb}")
                    for tb in range(TB)]
            yt_sb = [[apool.tile([128, 512], dt.bfloat16, name=f"yt{j}_{c}")
                      for c in range(QC)] for j in range(2)]

            # ---- QKV projections, chunk-interleaved ---------------------
            def qk_proj(w_k, wb_sb, dst, j, c):
                ps = ps_mm.tile([128, 512], dt.float32, tag="mm")
                for kt in range(KT):
                    nc.tensor.matmul(
                        ps[:],
                        w_k[kt][:, j * 128:(j + 1) * 128],
                        xt_kc[kt][c][:],
                        start=(kt == 0), stop=False,
                    )
                nc.tensor.matmul(
                    ps[:],
                    wb_sb[0:1, j * 128:(j + 1) * 128],
                    xb_sb[0:1, c * 512:(c + 1) * 512],
                    start=False, stop=True,
                )
                nc.vector.tensor_copy(dst[j][c][:], ps[:])

            def v_proj(tb):
                ps = ps_mm.tile([128, 512], dt.float32, tag="mm")
                psv = ps[:, :DVA]
                for kt in range(KT):
                    nc.tensor.matmul(
                        psv,
                        xt_kc[kt][tb // 4][:, (tb % 4) * 128:(tb % 4 + 1) * 128],
                        wv_k[kt][:],
                        start=(kt == 0), stop=False,
                    )
                nc.tensor.matmul(
                    psv,
                    xb_sb[0:1, tb * 128:(tb + 1) * 128],
                    wvb_sb[0:1, :],
                    start=False, stop=True,
                )
                nc.vector.tensor_copy(v_sb[tb][:], psv)

            # ---- attention: head pairs share the PE via row groups ------
            p_tiles = {}   # (hb, c) -> [[p_he0, p_he1] per jj]

            def out_proj(tb):
                for e in range(2):
                    ps = ps_mm.tile([128, 512], dt.float32, tag="mm")
                    for j in range(2):
                        nc.tensor.matmul(
                            ps[:],
                            yt_sb[j][tb // 4][:, (tb % 4) * 128:(tb % 4 + 1) * 128],
                            wp_sb[:, j, e * 512:(e + 1) * 512],
                            start=(j == 0), stop=(j == 1),
                        )
                    st = spool.tile([128, 512], dt.float32, tag="outstage")
                    nc.vector.tensor_copy(st[:], ps[:])
                    nc.sync.dma_start(
                        out[tb * 128:(tb + 1) * 128, e * 512:(e + 1) * 512], st[:]
                    )

            # ---- emission schedule: QKV chunk c enables attention chunk
            # c, which enables out-proj blocks 4c..4c+3.  Interleaving the
            # emission keeps ACT (exp) fed from ~10us in, overlapping the
            # PE-heavy QKV with the ACT-heavy attention. ------------------
            # ---- emission schedule ---------------------------------------
            # The PE stream is in-order, so its emission order must pace
            # S-pair production (consumed slowly by ACT exps) against
            # independent PE "filler" work: the next chunk's QKV and the
            # previous chunk's PV / out-proj.  Each S pair demands ~1.44us
            # of ACT; a pair itself is ~0.43us of PE, so ~1us of filler is
            # zipped in after each S pair.
            def s_pair(hb, c, jj, psy):
                qq0 = max(0, (jj - 4 * c) * 128)
                p_pair = []
                for he in range(2):
                    hp = he * 64
                    pss = ps_s.tile([128, 512], dt.float32, tag="s")
                    nc.tensor.matmul(
                        pss[:, qq0:512],
                        kt_sb[hb][jj // 4][hp:hp + 64,
                                           (jj % 4) * 128:(jj % 4 + 1) * 128],
                        qt_sb[hb][c][hp:hp + 64, qq0:512],
                        start=True, stop=True,
                    )
                    p = ppool.tile([128, 512], dt.bfloat16, tag="p")
                    if jj < 4 * c:
                        nc.scalar.activation(
                            p[:], pss[:],
                            mybir.ActivationFunctionType.Exp, scale=SCALE,
                        )
                    else:
                        if qq0 > 0:
                            nc.gpsimd.memset(p[:, 0:qq0], 0.0)
                        nc.scalar.activation(
                            p[:, qq0:512], pss[:, qq0:512],
                            mybir.ActivationFunctionType.Exp, scale=SCALE,
                        )
                        nc.gpsimd.tensor_tensor(
                            p[:, qq0:qq0 + 128], p[:, qq0:qq0 + 128],
                            mask_sb[:], mybir.AluOpType.mult,
                        )
                    p_pair.append(p)
                p_tiles[(hb, c)].append(p_pair)

            def pv_pair(hb, c, jj, psy):
                qq0 = max(0, (jj - 4 * c) * 128)
                jmax = 4 * c + 3
                for he in range(2):
                    h = 2 * hb + he
                    nc.tensor.matmul(
                        psy[he][:, qq0:512],
                        v_sb[jj][:, h * 65:h * 65 + 65],
                        p_tiles[(hb, c)][jj][he][:, qq0:512],
                        start=(jj == 0), stop=(jj == jmax),
                    )

            def pv_div(hb, c, psy):
                for he in range(2):
                    hp = he * 64
                    recip = spool.tile([1, 512], dt.bfloat16, tag="recip")
                    with nc.allow_low_precision(
                        reason="softmax denominators fit bf16"
                    ):
                        nc.vector.reciprocal(recip[:], psy[he][64:65, :])
                    psb = ps_bc.tile([64, 512], dt.float32, tag="bc")
                    nc.tensor.matmul(psb[:], ones64[0:1, :], recip[0:1, :],
                                     start=True, stop=True)
                    # DVE can read only one PSUM operand: stage the
                    # broadcast reciprocals in SBUF (bf16-exact copy)
                    bc_sb = spool.tile([64, 512], dt.bfloat16, tag="bcs")
                    nc.vector.tensor_copy(bc_sb[:], psb[:])
                    nc.vector.tensor_tensor(
                        yt_sb[hb][c][hp:hp + 64, :],
                        psy[he][0:64, :],
                        bc_sb[:],
                        mybir.AluOpType.mult,
                    )
                del p_tiles[(hb, c)]

            psy_live = {}

            def filler_units(c):
                """PE work independent of chunk c's exps: the next chunk's
                QKV and the chunk-before-last's out-proj, as
                (cost_us, closure, needed_by_block) units in
                dependency-safe order."""
                units = []
                if c == 0:
                    for tb in range(4):
                        units.append((1.0, lambda tb=tb: v_proj(tb), 0))
                if c + 1 < QC:
                    cn = c + 1
                    for j in range(2):
                        units.append((1.7, lambda j=j: qk_proj(
                            wq_k, wqb_sb, qt_sb, j, cn), cn))
                        units.append((1.7, lambda j=j: qk_proj(
                            wk_k, wkb_sb, kt_sb, j, cn), cn))
                    for tb in range(4 * cn, 4 * cn + 4):
                        units.append((1.0, lambda tb=tb: v_proj(tb), cn))
                if c >= 1:
                    cp = c - 1
                    for hb in range(2):
                        psy = psy_live.pop((hb, cp))
                        for jj in range(4 * cp + 4):
                            units.append((0.45, lambda hb=hb, jj=jj, psy=psy:
                                          pv_pair(hb, cp, jj, psy), None))
                        units.append((0.6, lambda hb=hb, psy=psy:
                                      pv_div(hb, cp, psy), None))
                    for tb in range(4 * cp, 4 * cp + 4):
                        units.append((0.9, lambda tb=tb: out_proj(tb), None))
                return units

            # chunk 0's Q/K runs before the pipelined blocks
            for j in range(2):
                qk_proj(wq_k, wqb_sb, qt_sb, j, 0)
                qk_proj(wk_k, wkb_sb, kt_sb, j, 0)

            LAG = 4  # (hb, jj) steps between an S pair and its PV pair

            for c in range(QC):
                for hb in range(2):
                    p_tiles[(hb, c)] = []
                    psy_live[(hb, c)] = [
                        ps_y.tile([65, 512], dt.float32, tag="y",
                                  name=f"psy{hb}_{c}_{he}")
                        for he in range(2)]
                fillers = filler_units(c)
                fi = 0
                deficit = 0.0
                for hb, jj in [(hb, jj) for jj in range(4 * c + 4)
                               for hb in range(2)]:
                    s_pair(hb, c, jj, None)
                    deficit += 0.1
                    while deficit > 0 and fi < len(fillers):
                        cost, fn, _ = fillers[fi]
                        fn()
                        deficit -= cost
                        fi += 1
                while fi < len(fillers):
                    fillers[fi][1]()
                    fi += 1

            # tail: PV + div + out-proj of the last chunk.  PV pairs are
            # interleaved (hb inner) to match the order the exps complete.
            cl = QC - 1
            psys = {hb: psy_live.pop((hb, cl)) for hb in range(2)}
            for jj in range(4 * cl + 4):
                for hb in range(2):
                    pv_pair(hb, cl, jj, psys[hb])
            for hb in range(2):
                pv_div(hb, cl, psys[hb])
            for tb in range(4 * cl, TB):
                out_proj(tb)

    if legalize:
        _split_excess_waits(nc)
    return nc


_NC_CACHE = None


def _get_nc():
    global _NC_CACHE
    if _NC_CACHE is None:
        _NC_CACHE = build_attention_nc()
    return _NC_CACHE


def _prep_core_inputs(x, Wq, bq, Wk, bk, Wv, bv, Wp, b, g):
    cols = slice(DHG * g, DHG * (g + 1))
    wv_aug = np.zeros((C, DVA), np.float32)
    wvb_aug = np.zeros((1, DVA), np.float32)
    for h in range(NH):
        wv_aug[:, 65 * h:65 * h + 64] = Wv[:, DHG * g + DH * h: DHG * g + DH * (h + 1)]
        wvb_aug[0, 65 * h:65 * h + 64] = bv[DHG * g + DH * h: DHG * g + DH * (h + 1)]
        wvb_aug[0, 65 * h + 64] = 1.0
    kk, qq = np.meshgrid(np.arange(128), np.arange(128), indexing="ij")
    mask = (kk <= qq).astype(np.float32)
    return {
        "xt": np.ascontiguousarray(x[b].T).astype(BF16),
        "xb": np.ones((1, T), BF16),
        "wq": np.ascontiguousarray(Wq[:, cols]).astype(BF16),
        "wk": np.ascontiguousarray(Wk[:, cols]).astype(BF16),
        "wv": wv_aug.astype(BF16),
        "wqb": bq[cols].reshape(1, DHG).astype(BF16),
        "wkb": bk[cols].reshape(1, DHG).astype(BF16),
        "wvb": wvb_aug.astype(BF16),
        "wp": np.ascontiguousarray(Wp[cols, :]).astype(BF16),
        "mask": mask.astype(BF16),
    }


def _run(x, Wq, bq, Wk, bk, Wv, bv, Wp, bp, **run_kwargs):
    from concourse.bass_utils import run_bass_kernel_spmd

    x = np.asarray(x, np.float32)
    args = tuple(np.asarray(a, np.float32) for a in (Wq, bq, Wk, bk, Wv, bv, Wp))
    bp = np.asarray(bp, np.float32)

    nc = _get_nc()
    in_maps = [
        _prep_core_inputs(x, *args, b=core // 4, g=core % 4) for core in range(8)
    ]
    res = run_bass_kernel_spmd(nc, in_maps, core_ids=list(range(8)), **run_kwargs)

    B = x.shape[0]
    out = np.zeros((B, T, C), np.float32)
    for core in range(8):
        out[core // 4] += res.results[core]["out"]
    out += bp[None, None, :]
    return out, res


def kernel(x, Wq, bq, Wk, bk, Wv, bv, Wp, bp):
    out, _ = _run(x, Wq, bq, Wk, bk, Wv, bv, Wp, bp)
    return out


if __name__ == "__main__":
    rng = np.random.default_rng(0)
    ins = {
        "x": rng.standard_normal((2, T, C), dtype=np.float32),
        "Wq": rng.standard_normal((C, C), dtype=np.float32) * 0.02,
        "bq": rng.standard_normal(C).astype(np.float32) * 0.02,
        "Wk": rng.standard_normal((C, C), dtype=np.float32) * 0.02,
        "bk": rng.standard_normal(C).astype(np.float32) * 0.02,
        "Wv": rng.standard_normal((C, C), dtype=np.float32) * 0.02,
        "bv": rng.standard_normal(C).astype(np.float32) * 0.02,
        "Wp": rng.standard_normal((C, C), dtype=np.float32) * 0.02,
        "bp": rng.standard_normal(C).astype(np.float32) * 0.02,
    }
    got = kernel(**ins)
    print("kernel ran, output shape", got.shape)



# revision 26
# speedup vs baseline: 1.3411x; 1.3411x over previous
"""Causal self-attention on 8 Trainium2 NeuronCores.

Sharding: core c = 4*b + g handles batch b (of 2) and head-group g (4 of 16
heads).  Weights are column-sliced (Wq/Wk/Wv) / row-sliced (Wp) per head
group; each core returns a partial output [T, C] in bf16 that the host
upcasts, sums per batch, and biases.

Per-core dataflow (all matmul operands bf16, fp32 PSUM accumulation):
  x^T loaded as [128, kt, t] chunk tiles.
  Q^T/K^T = W^T-slice @ x^T -> [128 d, 512 t] per (j, chunk); the per-d bias
    is folded into the PSUM->SBUF eviction (DVE tensor_scalar_add).
  V = x @ Wv_aug -> [t 128, 260] per t-block (aug: 65 cols/head, every 65th
    col is reset to 1.0 by a strided memset -> P@V also yields the softmax
    denominator in column 64 of each head's output).
  S^T[k,q] = K^T.T @ Q^T per (head-pair, k-block): two 64-contraction
    matmuls into one [128, 2, 512] PSUM tile, one merged exp (ACT) for both
    heads, diagonal blocks masked by a Pool multiply.
  PV reoriented [q, d]: y[q, 0:65] += P^T-slice.T @ V-slice per k-block
    (K=128, M=128, N=65 -- full PE array vs the [d, q] orientation's K=64).
    Softmax division is a per-partition reciprocal + tensor_scalar.
  y tiles are PE-transposed back to [d, q] (bv folded into the PSUM->SBUF
    eviction of the transpose), then out_partial = y^T.T @ Wp staged bf16
    and DMA'd per t-block.
"""

import re
import sys

sys.path.insert(0, "/opt/trn_rl_repo")

import numpy as np
import ml_dtypes

import bass_rust
import concourse.bass as bass
import concourse.mybir as mybir
from concourse.tile import TileContext
from concourse.vector_clock import ScopedClock

BF16 = ml_dtypes.bfloat16

T = 2048          # sequence length per batch
C = 1024          # model dim
DHG = 256         # head dims per core (4 heads x 64)
DH = 64           # head dim
NH = 4            # heads per core
DVA = NH * (DH + 1)  # 260: V augmented with a ones-column per head
KT = C // 128     # 8 full contraction tiles
TB = T // 128     # 16 row tiles
QC = T // 512     # 4 query chunks
SCALE = 0.125     # 1/sqrt(64)


class SplitDrainTileContext(TileContext):
    """Walrus TRN2 codegen rejects >4 sync waits on one instruction; the
    stock TileContext exit drain carries one wait per live proc.  Split them
    into single-wait drains chained on the sync sequencer."""

    def _drain_and_barrier(self, tick_clock, wait_clock):
        gc = tick_clock.global_clock
        ticks = [int(x) for x in re.findall(r"\d+", repr(gc))]
        for proc, tick in [(i, t) for i, t in enumerate(ticks) if t > 0]:
            sub = bass_rust.VectorClock()
            sub.require_at_least(proc, tick)
            inst = self.nc.sync.drain()
            wait_clock.add_sem_waits(inst.ins, ScopedClock({None: sub}))
        self.nc.sync.drain()
        self.nc.all_engine_barrier()
        assert self.sems is not None
        popped = self.nc._tile_sem_poison_stack.pop()
        assert popped is self._sem_poison
        self.nc.clear_and_free_semaphores(list(self.sems.allocated().values()))
        self.nc.all_engine_barrier()


def _split_excess_waits(nc, max_waits=1):
    """Walrus TRN2 codegen allows only ~2 sync waits per instruction.
    Hoist any excess onto same-engine InstNoOp carriers placed immediately
    before the instruction — the engine is in-order, so semantics are
    identical."""
    ctr = 0
    for fn in nc.m.functions:
        for bb in fn.blocks:
            new = []
            for inst in bb.instructions:
                si = inst.sync_info
                if (si and si.on_wait and len(si.on_wait) > max_waits
                        and "Unassigned" not in str(inst.engine)):
                    waits = list(si.on_wait)
                    for w in waits[:-max_waits]:
                        ctr += 1
                        nop = bass_rust.InstNoOp(
                            name=f"wsplit-{ctr}", ins=[], outs=[])
                        nop.engine = inst.engine
                        nop.sync_info = bass_rust.SyncInfo(
                            on_wait=[w], on_update=[])
                        new.append(nop)
                    si.on_wait = waits[-max_waits:]
                new.append(inst)
            bb.instructions = new


def build_attention_nc(legalize=True):
    nc = bass.Bass(num_devices=8)
    dt = mybir.dt

    xt = nc.dram_tensor("xt", [C, T], dt.bfloat16, kind="ExternalInput")
    wq = nc.dram_tensor("wq", [C, DHG], dt.bfloat16, kind="ExternalInput")
    wk = nc.dram_tensor("wk", [C, DHG], dt.bfloat16, kind="ExternalInput")
    wv = nc.dram_tensor("wv", [C, DVA], dt.bfloat16, kind="ExternalInput")
    wp = nc.dram_tensor("wp", [DHG, C], dt.bfloat16, kind="ExternalInput")
    # mask (x2, for the two heads of a pair) | 128x128 identity
    mi = nc.dram_tensor("mi", [128, 3 * 128], dt.bfloat16, kind="ExternalInput")
    # per-partition bias columns: bq(j0) bq(j1) bk(j0) bk(j1) bv(j0) bv(j1)
    bias = nc.dram_tensor("bias", [128, 6], dt.float32, kind="ExternalInput")
    out = nc.dram_tensor("out", [T, C], dt.bfloat16, kind="ExternalOutput")

    xt_r = xt.rearrange("(k p) t -> p k t", p=128)
    wq_r = wq.rearrange("(k p) d -> p k d", p=128)
    wk_r = wk.rearrange("(k p) d -> p k d", p=128)
    wv_r = wv.rearrange("(k p) d -> p k d", p=128)

    with SplitDrainTileContext(nc) as tc:
        with (
            tc.tile_pool(name="weights", bufs=1) as wpool,
            tc.tile_pool(name="acts", bufs=1) as apool,
            tc.tile_pool(name="ptiles", bufs=38) as ppool,
            tc.tile_pool(name="small", bufs=4) as spool,
            tc.tile_pool(name="ostage", bufs=3) as opool,
            tc.tile_pool(name="ps_mm", bufs=2, space="PSUM") as ps_mm,
            tc.tile_pool(name="ps_s", bufs=2, space="PSUM") as ps_s,
            tc.tile_pool(name="ps_pvt", bufs=2, space="PSUM") as ps_pvt,
        ):
            # ---- input loads: wq + xt chunk 0 first (both halved so the
            # first Q chain starts after 2 DMAs), then K/V weights, the
            # remaining x^T chunks, and Wp. -------------------------------
            mi_sb = wpool.tile([128, 3, 128], dt.bfloat16)
            bias_sb = wpool.tile([128, 6], dt.float32)
            wq_sb = [wpool.tile([128, 4, DHG], dt.bfloat16, name=f"wq{h}")
                     for h in range(2)]
            wk_sb = [wpool.tile([128, 4, DHG], dt.bfloat16, name=f"wk{h}")
                     for h in range(2)]
            wv_sb = [wpool.tile([128, 4, DVA], dt.bfloat16, name=f"wv{h}")
                     for h in range(2)]
            xt_c0 = [apool.tile([128, 4, 512], dt.bfloat16, name=f"xtc0_{h}")
                     for h in range(2)]
            xt_c = [apool.tile([128, KT, 512], dt.bfloat16, name=f"xtc{cc}")
                    for cc in range(1, QC)]
            wp_sb = wpool.tile([128, 2, C], dt.bfloat16)

            nc.sync.dma_start(xt_c0[0][:], xt_r[:, 0:4, 0:512])
            nc.sync.dma_start(wq_sb[0][:], wq_r[:, 0:4, :])
            nc.sync.dma_start(xt_c0[1][:], xt_r[:, 4:8, 0:512])
            nc.sync.dma_start(wq_sb[1][:], wq_r[:, 4:8, :])
            nc.sync.dma_start(mi_sb[:], mi.rearrange("p (g f) -> p g f", f=128))
            nc.sync.dma_start(bias_sb[:], bias[:])
            nc.sync.dma_start(wk_sb[0][:], wk_r[:, 0:4, :])
            nc.sync.dma_start(wk_sb[1][:], wk_r[:, 4:8, :])
            nc.sync.dma_start(wv_sb[0][:], wv_r[:, 0:4, :])
            nc.sync.dma_start(wv_sb[1][:], wv_r[:, 4:8, :])
            for cc in range(1, QC):
                nc.sync.dma_start(
                    xt_c[cc - 1][:], xt_r[:, :, cc * 512:(cc + 1) * 512])
            nc.sync.dma_start(wp_sb[:], wp.rearrange("(k p) d -> p k d", p=128))

            def xt_at(c, kt):
                if c == 0:
                    return xt_c0[kt // 4][:, kt % 4, :]
                return xt_c[c - 1][:, kt, :]

            def wqk_at(w_sb, kt):
                return w_sb[kt // 4][:, kt % 4, :]

            # SBUF activation tiles
            qt_sb = [[apool.tile([128, 512], dt.bfloat16, name=f"qt{j}_{c}")
                      for c in range(QC)] for j in range(2)]
            kt_sb = [[apool.tile([128, 512], dt.bfloat16, name=f"kt{j}_{c}")
                      for c in range(QC)] for j in range(2)]
            v_sb = [apool.tile([128, DVA], dt.bfloat16, name=f"v{tb}")
                    for tb in range(TB)]
            yt_sb = [[apool.tile([128, 512], dt.bfloat16, name=f"yt{j}_{c}")
                      for c in range(QC)] for j in range(2)]

            # ---- unit emitters ------------------------------------------
            def qk_chain(w_sb, dst, bcol, j, c):
                ps = ps_mm.tile([128, 512], dt.float32, tag="mm")
                for kt in range(KT):
                    nc.tensor.matmul(
                        ps[:],
                        wqk_at(w_sb, kt)[:, j * 128:(j + 1) * 128],
                        xt_at(c, kt),
                        start=(kt == 0), stop=(kt == KT - 1),
                    )
                nc.vector.tensor_scalar_add(
                    dst[j][c][:], ps[:], bias_sb[:, bcol:bcol + 1])

            def qk_half(w_sb, dst, bcol, j, c, half):
                """Self-contained half-width projection: full 8-ktile
                accumulation over 256 of the 512 t-columns + eviction, so
                arbitrary units may interleave without breaking a group."""
                ps = ps_mm.tile([128, 256], dt.float32, tag="mm",
                                name=f"qk{bcol}_{c}_{half}")
                t0 = half * 256
                for kt in range(KT):
                    nc.tensor.matmul(
                        ps[:],
                        wqk_at(w_sb, kt)[:, j * 128:(j + 1) * 128],
                        xt_at(c, kt)[:, t0:t0 + 256],
                        start=(kt == 0), stop=(kt == KT - 1),
                    )
                nc.vector.tensor_scalar_add(
                    dst[j][c][:, t0:t0 + 256], ps[:], bias_sb[:, bcol:bcol + 1])

            def v_half(tb, half):
                ps = ps_mm.tile([128, 130], dt.float32, tag="mm",
                                name=f"v{tb}_{half}")
                d0 = half * 130
                for kt in range(KT):
                    nc.tensor.matmul(
                        ps[:],
                        xt_at(tb // 4, kt)[:, (tb % 4) * 128:(tb % 4 + 1) * 128],
                        wv_sb[kt // 4][:, kt % 4, d0:d0 + 130],
                        start=(kt == 0), stop=(kt == KT - 1),
                    )
                nc.vector.tensor_copy(v_sb[tb][:, d0:d0 + 130], ps[:])
                # ones-columns for the softmax denominators
                nc.vector.memset(v_sb[tb][:, d0 + DH:d0 + 130:DH + 1], 1.0)

            p_tiles = {}   # (hb, c) -> list over jj of [128, 2, 512] tiles

            def s_pair(hb, c, jj):
                qq0 = max(0, (jj - 4 * c) * 128)
                pss = ps_s.tile([128, 2, 512], dt.float32, tag="s")
                for he in range(2):
                    hp = he * 64
                    nc.tensor.matmul(
                        pss[:, he, qq0:512],
                        kt_sb[hb][jj // 4][hp:hp + 64,
                                           (jj % 4) * 128:(jj % 4 + 1) * 128],
                        qt_sb[hb][c][hp:hp + 64, qq0:512],
                        start=True, stop=True,
                    )
                p = ppool.tile([128, 2, 512], dt.bfloat16, tag="p")
                nc.scalar.activation(
                    p[:, :, qq0:512], pss[:, :, qq0:512],
                    mybir.ActivationFunctionType.Exp, scale=SCALE,
                )
                if jj >= 4 * c:
                    nc.gpsimd.tensor_tensor(
                        p[:, :, qq0:qq0 + 128], p[:, :, qq0:qq0 + 128],
                        mi_sb[:, 0:2, :], mybir.AluOpType.mult,
                    )
                p_tiles[(hb, c)].append(p)

            y_cur = {}     # j -> y_sb tile for the q-block being reduced

            def pv_head(qb, h):
                c, qo = qb // 4, qb % 4
                hb, he = h // 2, h % 2
                ps = ps_pvt.tile([128, 512], dt.float32, tag="pvt",
                                 name=f"pv{qb}_{h}")
                for jj in range(qb + 1):
                    nc.tensor.matmul(
                        ps[:, 0:DH + 1],
                        p_tiles[(hb, c)][jj][:, he, qo * 128:(qo + 1) * 128],
                        v_sb[jj][:, h * 65:h * 65 + 65],
                        start=(jj == 0), stop=(jj == qb),
                    )
                r = spool.tile([128, 1], dt.float32, tag="r", name=f"r{qb}_{h}")
                nc.vector.reciprocal(r[:], ps[:, DH:DH + 1])
                if he == 0:
                    y_cur[(qb, hb)] = spool.tile(
                        [128, 128], dt.bfloat16, tag="y", bufs=8,
                        name=f"y{qb}_{hb}")
                nc.vector.tensor_scalar_mul(
                    y_cur[(qb, hb)][:, he * 64:(he + 1) * 64], ps[:, 0:DH], r[:])

            y_t = {}       # qb -> (y_sb j0, y_sb j1) awaiting transpose

            def transpose_j(qb, j):
                c, qo = qb // 4, qb % 4
                pst = ps_pvt.tile([128, 128], dt.bfloat16, tag="pvt",
                                  name=f"tp{qb}_{j}")
                nc.tensor.transpose(pst[:], y_t[qb][j][:], mi_sb[:, 2, :])
                nc.vector.tensor_scalar_add(
                    yt_sb[j][c][:, qo * 128:(qo + 1) * 128], pst[:],
                    bias_sb[:, 4 + j:5 + j])

            def op_half(tb, e, pool=None):
                pool = pool or ps_mm
                ps = pool.tile([128, 512], dt.float32, tag=pool.name[3:],
                               name=f"op{tb}_{e}")
                for j in range(2):
                    nc.tensor.matmul(
                        ps[:],
                        yt_sb[j][tb // 4][:, (tb % 4) * 128:(tb % 4 + 1) * 128],
                        wp_sb[:, j, e * 512:(e + 1) * 512],
                        start=(j == 0), stop=(j == 1),
                    )
                st = opool.tile([128, 512], dt.bfloat16, tag="st",
                                name=f"st{tb}_{e}")
                nc.vector.tensor_copy(st[:], ps[:])
                nc.sync.dma_start(
                    out[tb * 128:(tb + 1) * 128, e * 512:(e + 1) * 512],
                    st[:])

            # ---- emission schedule --------------------------------------
            # PE warmup: ~3us of dummy matmuls ramps the PE p-state to full
            # speed while the first DMAs are still in flight.
            warm_sb = wpool.tile([128, 128], dt.bfloat16)
            nc.vector.memset(warm_sb[:], 0.0)
            psw = ps_mm.tile([128, 128], dt.float32, tag="mm", name="warm")
            NWARM = 36
            for i in range(NWARM):
                nc.tensor.matmul(psw[:], warm_sb[:], warm_sb[:],
                                 start=(i == 0), stop=(i == NWARM - 1))

            # chunk 0's Q/K projections
            qk_chain(wq_sb, qt_sb, 0, 0, 0)
            qk_chain(wq_sb, qt_sb, 1, 1, 0)
            qk_chain(wk_sb, kt_sb, 2, 0, 0)
            qk_chain(wk_sb, kt_sb, 3, 1, 0)
            for hb in range(2):
                for c in range(QC):
                    p_tiles[(hb, c)] = []

            # Global flattened S stream: all (c, jj, hb) in order, zipped
            # with a single filler queue drained proportionally (plus
            # stall-covering) so PE stays fed and ACT (exp) never gates a
            # bunched region.  PE-time estimates pace the zip; correctness
            # only relies on emission order, the tile deps do the rest.
            s_units = [(c, jj, hb)
                       for c in range(QC)
                       for jj in range(4 * c + 4)
                       for hb in range(2)]
            NS = len(s_units)

            PE_CYC = 0.4166
            fillq = []        # (cost_ns, closure, kind, key)

            def push(cost, fn, kind=None, key=None, front=False):
                if front:
                    fillq.insert(0, (cost, fn, kind, key))
                else:
                    fillq.append((cost, fn, kind, key))

            # static fillers: chunk c's V projections + next chunk's Q/K,
            # in fine-grained self-contained units, ordered so deadlines
            # (QK before the chunk, V before pv) hold.
            def push_v(tb):
                for half in range(2):
                    push(433, (lambda tb=tb, h=half: v_half(tb, h)), "v", tb)

            def push_qk(w_sb, dst, bcol, j, cn):
                for half in range(2):
                    push(853, (lambda h=half: qk_half(w_sb, dst, bcol, j,
                                                      cn, h)), "qk", cn)

            for c in range(QC):
                push_v(4 * c)
                if c + 1 < QC:
                    push_qk(wq_sb, qt_sb, 0, 0, c + 1)
                push_v(4 * c + 1)
                if c + 1 < QC:
                    push_qk(wk_sb, kt_sb, 2, 0, c + 1)
                push_v(4 * c + 2)
                if c + 1 < QC:
                    push_qk(wq_sb, qt_sb, 1, 1, c + 1)
                push_v(4 * c + 3)
                if c + 1 < QC:
                    push_qk(wk_sb, kt_sb, 3, 1, c + 1)

            def pv_bundle_units(qb):
                units = []
                for h in range(NH):
                    units.append(((qb + 1) * 27, lambda qb=qb, h=h:
                                  pv_head(qb, h), "pv", qb))

                def snap(qb=qb):
                    y_t[qb] = (y_cur.pop((qb, 0)), y_cur.pop((qb, 1)))
                    transpose_j(qb, 0)
                units.append((53, snap, "t", qb))
                units.append((53, (lambda qb=qb: transpose_j(qb, 1)), "t", qb))
                for e in range(2):
                    units.append((426, (lambda tb=qb, e=e: op_half(tb, e)),
                                  "op", qb))
                return units

            done_fill = 0.0

            exp_end = []      # per s-unit: estimated exp completion
            est_pe = 0.0
            est_act = 0.0

            def drain_kind(kind, key):
                """Force-emit queued fillers of `kind` with key <= key."""
                nonlocal done_fill, est_pe
                rest = []
                for item in fillq:
                    if item[2] == kind and item[3] <= key:
                        item[1]()
                        done_fill += item[0]
                        est_pe += item[0]
                    else:
                        rest.append(item)
                fillq[:] = rest

            for n, (c, jj, hb) in enumerate(s_units):
                if jj == 0 and hb == 0 and c > 0:
                    drain_kind("qk", c)       # Q/K(c) must precede S(c)
                qq0 = max(0, (jj - 4 * c) * 128)
                s_cost = 2 * (512 - qq0) * PE_CYC
                # a ps_s bank recycles when exp (n-2) completes; spend
                # filler only to cover that stall, hold the rest in
                # reserve (leftovers are exp-independent tail work)
                bank_free = exp_end[n - 2] if n >= 2 else 0.0
                while est_pe < bank_free + 300 and fillq:
                    cost, fn, kind, key = fillq.pop(0)
                    fn()
                    est_pe += cost
                    done_fill += cost
                est_pe = max(est_pe, bank_free) + s_cost
                s_pair(hb, c, jj)
                est_act = max(est_act, est_pe + 150) + \
                    0.833 * 2 * (512 - qq0) + 200
                exp_end.append(est_act)
                # unlock pv for q-block qb once both its head-pairs' exps
                # are emitted plus one unit of lag; splice a static unit
                # between the head-pairs so the PSUM ping-pong (h2 reuses
                # h0's bank after its DVE eviction) never stalls PE.
                # out-proj goes to the BACK of the queue: it depends on
                # nothing downstream, so it is the stall reserve that
                # drains on demand (or at the tail).
                if n >= 1:
                    pc, pjj, phb = s_units[n - 1]
                    if phb == 1 and pjj >= 4 * pc:
                        qb = pjj
                        drain_kind("v", qb)   # V tiles feed pv directly
                        units = pv_bundle_units(qb)
                        op_units = units[-2:]
                        units = units[:-2]

                        def take_splice():
                            for i, item in enumerate(fillq):
                                if item[2] in ("v", "qk", "op"):
                                    return fillq.pop(i)
                            return None
                        # [h0 h1 X h2 h3 Y t0 t1]: X covers h2's bank
                        # reuse (waits h0's DVE eviction), Y covers t1
                        # (waits h3's division)
                        s1, s2 = take_splice(), take_splice()
                        if s2 is not None:
                            units = units[:4] + [s2] + units[4:]
                        if s1 is not None:
                            units = units[:2] + [s1] + units[2:]
                        for u in reversed(units):
                            push(*u[:2], kind=u[2], key=u[3], front=True)
                        for u in op_units:
                            push(*u[:2], kind=u[2], key=u[3])

            # tail: remaining queue, interleaved with the last q-block's
            # reduction so nothing serializes behind the final exps.  The
            # backlogged out-projs alternate between the two PSUM pools
            # (ps_pvt frees up as pv(15) retires) so the PSUM rotation
            # never gates back-to-back out-projs.
            tail = pv_bundle_units(TB - 1)[:-2]
            # fillq "op" entries are op-halves: reconstruct (tb, e) pairs
            op_halves = []
            seen = {}
            for _, _, kind, key in fillq:
                if kind == "op":
                    e = seen.get(key, 0)
                    seen[key] = e + 1
                    op_halves.append((key, e))
            for cost, fn, kind, _ in fillq:
                if kind != "op":
                    fn()
            oi = 0
            for u in tail:
                if oi < len(op_halves):
                    tb, e = op_halves[oi]
                    op_half(tb, e)
                    oi += 1
                u[1]()
            rest = op_halves[oi:] + [(TB - 1, 0), (TB - 1, 1)]
            for i, (tb, e) in enumerate(rest):
                pool = ps_pvt if (i // 2) % 2 == 0 else ps_mm
                op_half(tb, e, pool=pool)

    if legalize:
        _split_excess_waits(nc)
    return nc


_NC_CACHE = None


def _get_nc():
    global _NC_CACHE
    if _NC_CACHE is None:
        _NC_CACHE = build_attention_nc()
    return _NC_CACHE


def _prep_core_inputs(x, Wq, bq, Wk, bk, Wv, bv, Wp, b, g):
    cols = slice(DHG * g, DHG * (g + 1))
    wv_aug = np.zeros((C, DVA), np.float32)
    for h in range(NH):
        wv_aug[:, 65 * h:65 * h + 64] = \
            Wv[:, DHG * g + DH * h: DHG * g + DH * (h + 1)]
    kk, qq = np.meshgrid(np.arange(128), np.arange(128), indexing="ij")
    mask = (kk <= qq).astype(np.float32)
    mi = np.concatenate([mask, mask, np.eye(128, dtype=np.float32)], axis=1)
    bias = np.stack(
        [bq[cols][0:128], bq[cols][128:256],
         bk[cols][0:128], bk[cols][128:256],
         bv[cols][0:128], bv[cols][128:256]], axis=1)
    return {
        "xt": np.ascontiguousarray(x[b].T).astype(BF16),
        "wq": np.ascontiguousarray(Wq[:, cols]).astype(BF16),
        "wk": np.ascontiguousarray(Wk[:, cols]).astype(BF16),
        "wv": wv_aug.astype(BF16),
        "wp": np.ascontiguousarray(Wp[cols, :]).astype(BF16),
        "mi": mi.astype(BF16),
        "bias": np.ascontiguousarray(bias).astype(np.float32),
    }


def _run(x, Wq, bq, Wk, bk, Wv, bv, Wp, bp, **run_kwargs):
    from concourse.bass_utils import run_bass_kernel_spmd

    x = np.asarray(x, np.float32)
    args = tuple(np.asarray(a, np.float32) for a in (Wq, bq, Wk, bk, Wv, bv, Wp))
    bp = np.asarray(bp, np.float32)

    nc = _get_nc()
    in_maps = [
        _prep_core_inputs(x, *args, b=core // 4, g=core % 4) for core in range(8)
    ]
    res = run_bass_kernel_spmd(nc, in_maps, core_ids=list(range(8)), **run_kwargs)

    B = x.shape[0]
    out = np.zeros((B, T, C), np.float32)
    for core in range(8):
        out[core // 4] += np.asarray(res.results[core]["out"], np.float32)
    out += bp[None, None, :]
    return out, res


def kernel(x, Wq, bq, Wk, bk, Wv, bv, Wp, bp):
    out, _ = _run(x, Wq, bq, Wk, bk, Wv, bv, Wp, bp)
    return out


if __name__ == "__main__":
    rng = np.random.default_rng(0)
    ins = {
        "x": rng.standard_normal((2, T, C), dtype=np.float32),
        "Wq": rng.standard_normal((C, C), dtype=np.float32) * 0.02,
        "bq": rng.standard_normal(C).astype(np.float32) * 0.02,
        "Wk": rng.standard_normal((C, C), dtype=np.float32) * 0.02,
        "bk": rng.standard_normal(C).astype(np.float32) * 0.02,
        "Wv": rng.standard_normal((C, C), dtype=np.float32) * 0.02,
        "bv": rng.standard_normal(C).astype(np.float32) * 0.02,
        "Wp": rng.standard_normal((C, C), dtype=np.float32) * 0.02,
        "bp": rng.standard_normal(C).astype(np.float32) * 0.02,
    }
    got = kernel(**ins)
    print("kernel ran, output shape", got.shape)
